# revision 13
# baseline (speedup 1.0000x reference)
"""GraphSAGE supervised forward on 8 Trainium2 NeuronCores.

Full inputs in, full output out. Data-parallel over the B=1024 seed nodes:
128 seeds per core; neighbor rows shard as contiguous row ranges. Tiny
weights replicated.

v6 design — quantize + transpose on host, PE group-sums, algebraic fold:
  - hop-2 neighbors (the 82MB/core f32 stream) are sent as fp8e4m3 in
    feat-major, PHASE-MAJOR-per-tile layout; hop-1 neighbors as bf16
    [128, 3200]; seeds f32. End-to-end max rel err ~2e-3 (gate 2e-2):
    the two mean-over-25 stages attenuate per-element quantization noise.
  - group-sum of 25 phases runs on the PE as accumulating identity
    matmuls: stationary [I;I] fp8 + DoubleRow packs 2 phases per
    column-slot (12 DR + 1 plain matmul per tile, all moving operands
    contiguous blocks), f32 PSUM accumulation.
  - key fold: the hop-1 mean commutes with the aggregator matmul, so
    per-column hidden states are never materialized. Only per-seed sums
    are kept: redS = DVE reduce of the GS PSUM (25 group-cols -> seed),
    negS = DVE group-sum of negT. Then per side
        m1T  = wtop^T negS + wbot^T redS      (25x mean_j h_j, transposed)
        oT   = wtop^T seedT + wbot^T m1T      (hop-1 output)
    with wbot pre-scaled by 1/25 on host. This removes the per-tile hT
    matmuls + copies that serialized the v3-v5 pipelines.
  - hop-1 + 4-layer MLP + softmax (f32) in 4 parts as seed ranges
    complete; sides interleave per tile; ragged tiles (small first/last)
    cut pipeline fill and tail latency.
"""

import sys

for _p in ("/opt/trn_rl_repo", "/root/.axon_site/_ro/trn_rl_repo"):
    if _p not in sys.path:
        sys.path.append(_p)

import numpy as np
import ml_dtypes
from contextlib import ExitStack

import concourse.bass as bass
import concourse.tile as tile
from concourse import bacc, mybir
from concourse.bass_utils import run_bass_kernel_spmd

B, S, D = 1024, 25, 128
NCORES = 8
BL = B // NCORES          # 128 seeds per core
G1 = BL * S               # 3200 hop-1 rows (= hop-2 groups) per core
G2 = G1 * S               # 80000 hop-2 rows per core

# ragged stream tiles (groups per tile, per side); sum = G1
SIZES = [100, 200, 300, 400, 400, 400, 400, 400, 300, 200, 100]
OFFS = np.cumsum([0] + SIZES).tolist()
NTT = len(SIZES)
assert OFFS[-1] == G1 and all(sz % S == 0 for sz in SIZES)

F32 = mybir.dt.float32
BF16 = mybir.dt.bfloat16
F8 = mybir.dt.float8e4
AX = mybir.AxisListType
AF = mybir.ActivationFunctionType
DR = mybir.MatmulPerfMode.DoubleRow

NPF8 = ml_dtypes.float8_e4m3
NPBF = ml_dtypes.bfloat16


def _build_program():
    nc = bacc.Bacc("TRN2", target_bir_lowering=False, debug=False)

    ins = {}
    for side in ("s", "d"):
        ins[f"seed_{side}"] = nc.dram_tensor(f"seed_{side}", [D, BL], F32, kind="ExternalInput")
        ins[f"neg_{side}"] = nc.dram_tensor(f"neg_{side}", [D, G1], BF16, kind="ExternalInput")
        ins[f"nn_{side}"] = nc.dram_tensor(f"nn_{side}", [D, G2], F8, kind="ExternalInput")
    for name, shape, dt in (
        ("ident2", [D, 2 * D], F8),
        ("wtop32", [D, D], F32), ("wbot32", [D, D], F32),
        ("w1t", [D, D], F32), ("w1b", [D, D], F32),
        ("w2m", [D, 64], F32), ("w3m", [64, 8], F32), ("w4m", [8, 2], F32),
    ):
        ins[name] = nc.dram_tensor(name, shape, dt, kind="ExternalInput")
    out_dram = nc.dram_tensor("out", [BL, 2], F32, kind="ExternalOutput")

    with tile.TileContext(nc) as tc, ExitStack() as ctx:
        const = ctx.enter_context(tc.tile_pool(name="const", bufs=1))
        persist = ctx.enter_context(tc.tile_pool(name="persist", bufs=1))
        stream = ctx.enter_context(tc.tile_pool(name="stream", bufs=6))
        work = ctx.enter_context(tc.tile_pool(name="work", bufs=3))
        psA = ctx.enter_context(tc.tile_pool(name="psA", bufs=4, space="PSUM"))
        psM = ctx.enter_context(tc.tile_pool(name="psM", bufs=2, space="PSUM"))

        def load_const(name, shape, dt):
            t = const.tile(shape, dt, tag=name, name=name)
            nc.gpsimd.dma_start(t[:], ins[name].ap())
            return t

        # order matters: ident2 feeds the first stream tile's matmuls;
        # the big negT loads go last so they don't starve early stream tiles
        ident2 = load_const("ident2", [D, 2 * D], F8)
        negT, seedT = {}, {}
        for side in ("s", "d"):
            seedT[side] = const.tile([D, BL], F32, tag=f"seedT_{side}", name=f"seedT_{side}")
            nc.gpsimd.dma_start(seedT[side][:], ins[f"seed_{side}"].ap())
        wtop32 = load_const("wtop32", [D, D], F32)
        wbot32 = load_const("wbot32", [D, D], F32)
        w1t = load_const("w1t", [D, D], F32)
        w1b = load_const("w1b", [D, D], F32)
        w2m = load_const("w2m", [D, 64], F32)
        w3m = load_const("w3m", [64, 8], F32)
        w4m = load_const("w4m", [8, 2], F32)
        for side in ("s", "d"):
            negT[side] = const.tile([D, G1], BF16, tag=f"negT_{side}", name=f"negT_{side}")
            nc.gpsimd.dma_start(negT[side][:], ins[f"neg_{side}"].ap())

        idv2 = ident2.rearrange("p (j m) -> p j m", j=2)  # [128, 2, 128]
        id1 = ident2[:, 0:D]                              # [128, 128]

        negS, redS = {}, {}
        for side in ("s", "d"):
            negS[side] = persist.tile([D, BL], F32, tag=f"negS_{side}", name=f"negS_{side}")
            redS[side] = persist.tile([D, BL], F32, tag=f"redS_{side}", name=f"redS_{side}")

        oT = {}
        PARTS = [(0, 40), (40, 72), (72, 104), (104, BL)]

        def hop1_m1(side, pi):
            lo, hi = PARTS[pi]
            w = hi - lo
            # m1T = wtop^T negS + wbot^T redS  (= 25x transposed mean_j h_j)
            ps_m = psM.tile([D, w], F32, tag="ps_m")
            nc.tensor.matmul(ps_m[:], wtop32[:], negS[side][:, lo:hi], start=True, stop=False)
            nc.tensor.matmul(ps_m[:], wbot32[:], redS[side][:, lo:hi], start=False, stop=True)
            m1 = work.tile([D, w], F32, tag=f"m1_{side}")
            nc.scalar.activation(m1[:], ps_m[:], AF.Copy)
            oT[side, pi, "m1"] = m1

        def hop1_o(side, pi):
            lo, hi = PARTS[pi]
            w = hi - lo
            # oT = wtop^T seedT + wbot^T m1T
            ps_o = psM.tile([D, w], F32, tag="ps_m")
            nc.tensor.matmul(ps_o[:], wtop32[:], seedT[side][:, lo:hi], start=True, stop=False)
            nc.tensor.matmul(ps_o[:], wbot32[:], oT[side, pi, "m1"][:], start=False, stop=True)
            ot = persist.tile([D, w], F32, tag=f"oT_{side}{pi}")
            nc.scalar.activation(ot[:], ps_o[:], AF.Copy)
            oT[side, pi] = ot

        mst = {}

        def mlp_l1(pi):
            lo, hi = PARTS[pi]
            w = hi - lo
            ps1 = psM.tile([D, w], F32, tag="ps_m")
            nc.tensor.matmul(ps1[:], w1t[:], oT["s", pi][:], start=True, stop=False)
            nc.tensor.matmul(ps1[:], w1b[:], oT["d", pi][:], start=False, stop=True)
            h1 = work.tile([D, w], F32, tag="h1")
            nc.scalar.activation(h1[:], ps1[:], AF.Relu)
            mst[pi, 1] = h1

        def mlp_l23(pi):
            lo, hi = PARTS[pi]
            w = hi - lo
            ps2 = psM.tile([64, w], F32, tag="ps_m")
            nc.tensor.matmul(ps2[:], w2m[:], mst[pi, 1][:])
            h2 = work.tile([64, w], F32, tag="h2")
            nc.scalar.activation(h2[:], ps2[:], AF.Relu)
            ps3 = psM.tile([8, w], F32, tag="ps_m")
            nc.tensor.matmul(ps3[:], w3m[:], h2[:])
            h3 = work.tile([8, w], F32, tag="h3")
            nc.scalar.activation(h3[:], ps3[:], AF.Relu)
            mst[pi, 3] = h3

        def mlp_sm(pi):
            lo, hi = PARTS[pi]
            w = hi - lo
            ps4 = psM.tile([w, 2], F32, tag="ps_m")
            nc.tensor.matmul(ps4[:], mst[pi, 3][:], w4m[:])
            lg = work.tile([w, 2], F32, tag="lg")
            nc.scalar.activation(lg[:], ps4[:], AF.Copy)
            nm = work.tile([w, 1], F32, tag="nm")
            nc.vector.reduce_max(nm[:], lg[:], axis=AX.X, negate=True)
            ex = work.tile([w, 2], F32, tag="ex")
            se = work.tile([w, 1], F32, tag="se")
            nc.scalar.activation(ex[:], lg[:], AF.Exp, bias=nm[:], accum_out=se[:])
            rc = work.tile([w, 1], F32, tag="rc")
            nc.vector.reciprocal(rc[:], se[:])
            o = work.tile([w, 2], F32, tag="o")
            nc.vector.tensor_scalar_mul(o[:], ex[:], rc[:])
            # SWDGE: a sync-queue store would head-of-line block stream loads
            nc.gpsimd.dma_start(out_dram.ap()[lo:hi], o[:])

        seeds_done = {"s": 0, "d": 0}
        next_part = [0]
        pieces = []

        def maybe_parts():
            # enqueue part pieces once both sides' seed sums reach a boundary;
            # pieces are drained 2 per stream tile so the in-order PE queue
            # never sits behind a long dependent chain
            while next_part[0] < len(PARTS) and min(seeds_done.values()) >= PARTS[next_part[0]][1]:
                pi = next_part[0]
                pieces.extend([
                    lambda p=pi: hop1_m1("s", p),
                    lambda p=pi: hop1_m1("d", p),
                    lambda p=pi: hop1_o("s", p),
                    lambda p=pi: hop1_o("d", p),
                    lambda p=pi: mlp_l1(p),
                    lambda p=pi: mlp_l23(p),
                    lambda p=pi: mlp_sm(p),
                ])
                next_part[0] += 1

        def drain_pieces(k):
            for _ in range(k):
                if pieces:
                    pieces.pop(0)()

        def stream_tile(side, t):
            g0, sz = OFFS[t], SIZES[t]
            xt = stream.tile([D, sz * S], F8, tag="xt", name="xt")
            nc.sync.dma_start(xt[:], ins[f"nn_{side}"].ap()[:, g0 * S:(g0 + sz) * S])
            # phase-major tile: xr[:, k, :] = phase k's sz group-columns
            xr = xt.rearrange("p (k g) -> p k g", k=S)
            ps = psA.tile([D, sz], F32, tag="ps_red")
            for i in range(S // 2):
                nc.tensor.matmul(
                    ps[:], idv2, xr[:, 2 * i:2 * i + 2, :],
                    start=(i == 0), stop=False, perf_mode=DR,
                )
            nc.tensor.matmul(ps[:], id1, xr[:, S - 1, :], start=False, stop=True)
            # per-seed sums straight from PSUM (25 group-cols per seed)
            nc.vector.reduce_sum(
                redS[side][:, g0 // S:(g0 + sz) // S],
                ps.rearrange("p (b s) -> p b s", s=S),
                axis=AX.X,
            )
            seeds_done[side] = (g0 + sz) // S

        for t in range(NTT):
            for side in ("s", "d"):
                stream_tile(side, t)
                if t == 1:
                    # per-side group-sum of negT while the pipeline fills
                    nc.vector.reduce_sum(
                        negS[side][:],
                        negT[side].rearrange("p (b s) -> p b s", s=S),
                        axis=AX.X,
                    )
                maybe_parts()
                drain_pieces(2)
        while pieces:
            drain_pieces(1)

    nc.compile()
    return nc


_NC_CACHE = None


def _get_program():
    global _NC_CACHE
    if _NC_CACHE is None:
        _NC_CACHE = _build_program()
    return _NC_CACHE


def kernel(src, src_neg, src_neg_neg, dst, dst_neg, dst_neg_neg, w2, W1, W2, W3, W4,
           _trace=False, **trace_kwargs):
    nc = _get_program()

    w2 = np.asarray(w2, np.float32)
    W1 = np.asarray(W1, np.float32)
    wtop = np.ascontiguousarray(w2[:D])
    wbot = np.ascontiguousarray(w2[D:]) / np.float32(S)
    eye = np.eye(D, dtype=np.float32)
    rep = {
        "ident2": np.concatenate([eye, eye], axis=1).astype(NPF8),
        "wtop32": wtop, "wbot32": wbot,
        "w1t": np.ascontiguousarray(W1[:D]),
        "w1b": np.ascontiguousarray(W1[D:]),
        "w2m": np.asarray(W2, np.float32),
        "w3m": np.asarray(W3, np.float32),
        "w4m": np.asarray(W4, np.float32),
    }

    def shardT(x, dt, rows):
        # [NCORES*rows, D] -> transposed per core -> [NCORES, D, rows]
        return np.ascontiguousarray(
            np.asarray(x).astype(dt).reshape(NCORES, rows, D).transpose(0, 2, 1)
        )

    def shard_nn(x):
        # [NCORES*G2, D] -> fp8, feat-major + phase-major per ragged tile:
        # out[c, f, OFFS[t]*S + k*SIZES[t] + g] = x[c*G2 + (OFFS[t]+g)*S + k, f]
        x8 = np.asarray(x).astype(NPF8).reshape(NCORES, G1, S, D)
        out = np.empty((NCORES, D, G2), NPF8)
        for t, sz in enumerate(SIZES):
            g0 = OFFS[t]
            blk = x8[:, g0:g0 + sz]                    # [C, sz, S, D]
            out[:, :, g0 * S:(g0 + sz) * S] = (
                blk.transpose(0, 3, 2, 1).reshape(NCORES, D, sz * S)
            )
        return out

    big = {
        "nn_s": shard_nn(src_neg_neg),
        "nn_d": shard_nn(dst_neg_neg),
        "neg_s": shardT(src_neg, NPBF, G1),
        "neg_d": shardT(dst_neg, NPBF, G1),
        "seed_s": shardT(src, np.float32, BL),
        "seed_d": shardT(dst, np.float32, BL),
    }
    in_maps = []
    for c in range(NCORES):
        m = dict(rep)
        for k, v in big.items():
            m[k] = v[c]
        in_maps.append(m)

    res = run_bass_kernel_spmd(
        nc, in_maps, list(range(NCORES)), trace=_trace, **trace_kwargs
    )
    out = np.concatenate([res.results[c]["out"] for c in range(NCORES)], axis=0)
    if _trace:
        return out, res
    return out


# revision 14
# speedup vs baseline: 1.1443x; 1.1443x over previous
"""GraphSAGE supervised forward on 8 Trainium2 NeuronCores.

Full inputs in, full output out. Data-parallel over the B=1024 seed nodes:
128 seeds per core; neighbor rows shard as contiguous row ranges. Tiny
weights replicated.

v6 design — quantize + transpose on host, PE group-sums, algebraic fold:
  - hop-2 neighbors (the 82MB/core f32 stream) are sent as fp8e4m3 in
    feat-major, PHASE-MAJOR-per-tile layout; hop-1 neighbors as bf16
    [128, 3200]; seeds f32. End-to-end max rel err ~2e-3 (gate 2e-2):
    the two mean-over-25 stages attenuate per-element quantization noise.
  - group-sum of 25 phases runs on the PE as accumulating identity
    matmuls: stationary [I;I] fp8 + DoubleRow packs 2 phases per
    column-slot (12 DR + 1 plain matmul per tile, all moving operands
    contiguous blocks), f32 PSUM accumulation.
  - key fold: the hop-1 mean commutes with the aggregator matmul, so
    per-column hidden states are never materialized. Only per-seed sums
    are kept: redS = DVE reduce of the GS PSUM (25 group-cols -> seed),
    negS = DVE group-sum of negT. Then per side
        m1T  = wtop^T negS + wbot^T redS      (25x mean_j h_j, transposed)
        oT   = wtop^T seedT + wbot^T m1T      (hop-1 output)
    with wbot pre-scaled by 1/25 on host. This removes the per-tile hT
    matmuls + copies that serialized the v3-v5 pipelines.
  - hop-1 + 4-layer MLP + softmax (f32) in 4 parts as seed ranges
    complete; sides interleave per tile; ragged tiles (small first/last)
    cut pipeline fill and tail latency.
"""

import sys

for _p in ("/opt/trn_rl_repo", "/root/.axon_site/_ro/trn_rl_repo"):
    if _p not in sys.path:
        sys.path.append(_p)

import numpy as np
import ml_dtypes
from contextlib import ExitStack

import concourse.bass as bass
import concourse.tile as tile
from concourse import bacc, mybir
from concourse.bass_utils import run_bass_kernel_spmd

B, S, D = 1024, 25, 128
NCORES = 8
BL = B // NCORES          # 128 seeds per core
G1 = BL * S               # 3200 hop-1 rows (= hop-2 groups) per core
G2 = G1 * S               # 80000 hop-2 rows per core

# ragged stream tiles (groups per tile, per side); sum = G1
SIZES = [100, 400, 400, 400, 400, 400, 400, 400, 200, 100]
OFFS = np.cumsum([0] + SIZES).tolist()
NTT = len(SIZES)
assert OFFS[-1] == G1 and all(sz % S == 0 for sz in SIZES)

F32 = mybir.dt.float32
F16 = mybir.dt.float16
BF16 = mybir.dt.bfloat16
F8 = mybir.dt.float8e4
AX = mybir.AxisListType
AF = mybir.ActivationFunctionType
DR = mybir.MatmulPerfMode.DoubleRow

NPF8 = ml_dtypes.float8_e4m3
NPBF = ml_dtypes.bfloat16
NPF16 = np.float16


def _build_program():
    nc = bacc.Bacc("TRN2", target_bir_lowering=False, debug=False)

    ins = {}
    for side in ("s", "d"):
        ins[f"seed_{side}"] = nc.dram_tensor(f"seed_{side}", [D, BL], F16, kind="ExternalInput")
        ins[f"neg_{side}"] = nc.dram_tensor(f"neg_{side}", [D, G1], BF16, kind="ExternalInput")
        ins[f"nn_{side}"] = nc.dram_tensor(f"nn_{side}", [D, G2], F8, kind="ExternalInput")
    for name, shape, dt in (
        ("ident2", [D, 2 * D], F8),
        ("wtopH", [D, D], F16), ("wbotH", [D, D], F16),
        ("w1t", [D, D], F16), ("w1b", [D, D], F16),
        ("w2m", [D, 64], F16), ("w3m", [64, 8], F16), ("w4m", [8, 2], F16),
    ):
        ins[name] = nc.dram_tensor(name, shape, dt, kind="ExternalInput")
    out_dram = nc.dram_tensor("out", [BL, 2], F32, kind="ExternalOutput")

    with tile.TileContext(nc) as tc, ExitStack() as ctx:
        const = ctx.enter_context(tc.tile_pool(name="const", bufs=1))
        persist = ctx.enter_context(tc.tile_pool(name="persist", bufs=1))
        stream = ctx.enter_context(tc.tile_pool(name="stream", bufs=6))
        work = ctx.enter_context(tc.tile_pool(name="work", bufs=3))
        psA = ctx.enter_context(tc.tile_pool(name="psA", bufs=4, space="PSUM"))
        psM = ctx.enter_context(tc.tile_pool(name="psM", bufs=2, space="PSUM"))

        def load_const(name, shape, dt):
            t = const.tile(shape, dt, tag=name, name=name)
            nc.gpsimd.dma_start(t[:], ins[name].ap())
            return t

        # order matters: ident2 feeds the first stream tile's matmuls;
        # the big negT loads go last so they don't starve early stream tiles
        ident2 = load_const("ident2", [D, 2 * D], F8)
        negT, seedT = {}, {}
        for side in ("s", "d"):
            seedT[side] = const.tile([D, BL], F16, tag=f"seedT_{side}", name=f"seedT_{side}")
            nc.gpsimd.dma_start(seedT[side][:], ins[f"seed_{side}"].ap())
        wtopH = load_const("wtopH", [D, D], F16)
        wbotH = load_const("wbotH", [D, D], F16)
        w1t = load_const("w1t", [D, D], F16)
        w1b = load_const("w1b", [D, D], F16)
        w2m = load_const("w2m", [D, 64], F16)
        w3m = load_const("w3m", [64, 8], F16)
        w4m = load_const("w4m", [8, 2], F16)
        for side in ("s", "d"):
            negT[side] = const.tile([D, G1], BF16, tag=f"negT_{side}", name=f"negT_{side}")
            nc.gpsimd.dma_start(negT[side][:], ins[f"neg_{side}"].ap())

        idv2 = ident2.rearrange("p (j m) -> p j m", j=2)  # [128, 2, 128]
        id1 = ident2[:, 0:D]                              # [128, 128]

        negS, redS = {}, {}
        for side in ("s", "d"):
            negS[side] = persist.tile([D, BL], F16, tag=f"negS_{side}", name=f"negS_{side}")
            redS[side] = persist.tile([D, BL], F16, tag=f"redS_{side}", name=f"redS_{side}")

        oT = {}
        PARTS = [(0, 52), (52, 84), (84, 116), (116, BL)]

        def hop1_m1(side, pi):
            lo, hi = PARTS[pi]
            w = hi - lo
            # m1T = wtop^T negS + wbot^T redS  (= 25x transposed mean_j h_j)
            ps_m = psM.tile([D, w], F32, tag="ps_m")
            nc.tensor.matmul(ps_m[:], wtopH[:], negS[side][:, lo:hi], start=True, stop=False)
            nc.tensor.matmul(ps_m[:], wbotH[:], redS[side][:, lo:hi], start=False, stop=True)
            m1 = work.tile([D, w], F16, tag=f"m1_{side}")
            nc.scalar.activation(m1[:], ps_m[:], AF.Copy)
            oT[side, pi, "m1"] = m1

        def hop1_o(side, pi):
            lo, hi = PARTS[pi]
            w = hi - lo
            # oT = wtop^T seedT + wbot^T m1T
            ps_o = psM.tile([D, w], F32, tag="ps_m")
            nc.tensor.matmul(ps_o[:], wtopH[:], seedT[side][:, lo:hi], start=True, stop=False)
            nc.tensor.matmul(ps_o[:], wbotH[:], oT[side, pi, "m1"][:], start=False, stop=True)
            ot = persist.tile([D, w], F16, tag=f"oT_{side}{pi}")
            nc.scalar.activation(ot[:], ps_o[:], AF.Copy)
            oT[side, pi] = ot

        mst = {}

        def mlp_l1(pi):
            lo, hi = PARTS[pi]
            w = hi - lo
            ps1 = psM.tile([D, w], F32, tag="ps_m")
            nc.tensor.matmul(ps1[:], w1t[:], oT["s", pi][:], start=True, stop=False)
            nc.tensor.matmul(ps1[:], w1b[:], oT["d", pi][:], start=False, stop=True)
            h1 = work.tile([D, w], F16, tag="h1")
            nc.scalar.activation(h1[:], ps1[:], AF.Relu)
            mst[pi, 1] = h1

        def mlp_l23(pi):
            lo, hi = PARTS[pi]
            w = hi - lo
            ps2 = psM.tile([64, w], F32, tag="ps_m")
            nc.tensor.matmul(ps2[:], w2m[:], mst[pi, 1][:])
            h2 = work.tile([64, w], F16, tag="h2")
            nc.scalar.activation(h2[:], ps2[:], AF.Relu)
            ps3 = psM.tile([8, w], F32, tag="ps_m")
            nc.tensor.matmul(ps3[:], w3m[:], h2[:])
            h3 = work.tile([8, w], F16, tag="h3")
            nc.scalar.activation(h3[:], ps3[:], AF.Relu)
            mst[pi, 3] = h3

        def mlp_sm(pi):
            lo, hi = PARTS[pi]
            w = hi - lo
            ps4 = psM.tile([w, 2], F32, tag="ps_m")
            nc.tensor.matmul(ps4[:], mst[pi, 3][:], w4m[:])
            lg = work.tile([w, 2], F32, tag="lg")
            nc.scalar.activation(lg[:], ps4[:], AF.Copy)
            nm = work.tile([w, 1], F32, tag="nm")
            nc.vector.reduce_max(nm[:], lg[:], axis=AX.X, negate=True)
            ex = work.tile([w, 2], F32, tag="ex")
            se = work.tile([w, 1], F32, tag="se")
            nc.scalar.activation(ex[:], lg[:], AF.Exp, bias=nm[:], accum_out=se[:])
            rc = work.tile([w, 1], F32, tag="rc")
            nc.vector.reciprocal(rc[:], se[:])
            o = work.tile([w, 2], F32, tag="o")
            nc.vector.tensor_scalar_mul(o[:], ex[:], rc[:])
            # SWDGE: a sync-queue store would head-of-line block stream loads
            nc.gpsimd.dma_start(out_dram.ap()[lo:hi], o[:])

        seeds_done = {"s": 0, "d": 0}
        next_part = [0]
        pieces = []

        def maybe_parts():
            # enqueue part pieces once both sides' seed sums reach a boundary;
            # pieces are drained 2 per stream tile so the in-order PE queue
            # never sits behind a long dependent chain
            while next_part[0] < len(PARTS) and min(seeds_done.values()) >= PARTS[next_part[0]][1]:
                pi = next_part[0]
                pieces.extend([
                    lambda p=pi: hop1_m1("s", p),
                    lambda p=pi: hop1_m1("d", p),
                    lambda p=pi: hop1_o("s", p),
                    lambda p=pi: hop1_o("d", p),
                    lambda p=pi: mlp_l1(p),
                    lambda p=pi: mlp_l23(p),
                    lambda p=pi: mlp_sm(p),
                ])
                next_part[0] += 1

        def drain_pieces(k):
            for _ in range(k):
                if pieces:
                    pieces.pop(0)()

        def stream_tile(side, t):
            g0, sz = OFFS[t], SIZES[t]
            xt = stream.tile([D, sz * S], F8, tag="xt", name="xt")
            nc.sync.dma_start(xt[:], ins[f"nn_{side}"].ap()[:, g0 * S:(g0 + sz) * S])
            # phase-major tile: xr[:, k, :] = phase k's sz group-columns
            xr = xt.rearrange("p (k g) -> p k g", k=S)
            ps = psA.tile([D, sz], F32, tag="ps_red")
            for i in range(S // 2):
                nc.tensor.matmul(
                    ps[:], idv2, xr[:, 2 * i:2 * i + 2, :],
                    start=(i == 0), stop=False, perf_mode=DR,
                )
            nc.tensor.matmul(ps[:], id1, xr[:, S - 1, :], start=False, stop=True)
            # per-seed sums straight from PSUM (25 group-cols per seed);
            # fp16 out: one rounding of an f32 sum, feeds the 5x-attenuated
            # mean half of hop-1
            with nc.allow_low_precision(reason="fp16 out of f32 psum sums"):
                nc.vector.reduce_sum(
                    redS[side][:, g0 // S:(g0 + sz) // S],
                    ps.rearrange("p (b s) -> p b s", s=S),
                    axis=AX.X,
                )
            seeds_done[side] = (g0 + sz) // S

        for t in range(NTT):
            for side in ("s", "d"):
                stream_tile(side, t)
                if t == 1:
                    # per-side group-sum of negT while the pipeline fills
                    with nc.allow_low_precision(reason="fp16 out of bf16 sums"):
                        nc.vector.reduce_sum(
                            negS[side][:],
                            negT[side].rearrange("p (b s) -> p b s", s=S),
                            axis=AX.X,
                        )
                maybe_parts()
                drain_pieces(2)
        while pieces:
            drain_pieces(1)

    nc.compile()
    return nc


_NC_CACHE = None


def _get_program():
    global _NC_CACHE
    if _NC_CACHE is None:
        _NC_CACHE = _build_program()
    return _NC_CACHE


def kernel(src, src_neg, src_neg_neg, dst, dst_neg, dst_neg_neg, w2, W1, W2, W3, W4,
           _trace=False, **trace_kwargs):
    nc = _get_program()

    w2 = np.asarray(w2, np.float32)
    W1 = np.asarray(W1, np.float32)
    wtop = np.ascontiguousarray(w2[:D])
    wbot = np.ascontiguousarray(w2[D:]) / np.float32(S)
    eye = np.eye(D, dtype=np.float32)
    rep = {
        "ident2": np.concatenate([eye, eye], axis=1).astype(NPF8),
        "wtopH": wtop.astype(NPF16), "wbotH": wbot.astype(NPF16),
        "w1t": np.ascontiguousarray(W1[:D]).astype(NPF16),
        "w1b": np.ascontiguousarray(W1[D:]).astype(NPF16),
        "w2m": np.asarray(W2, NPF16),
        "w3m": np.asarray(W3, NPF16),
        "w4m": np.asarray(W4, NPF16),
    }

    def shardT(x, dt, rows):
        # [NCORES*rows, D] -> transposed per core -> [NCORES, D, rows]
        return np.ascontiguousarray(
            np.asarray(x).astype(dt).reshape(NCORES, rows, D).transpose(0, 2, 1)
        )

    def shard_nn(x):
        # [NCORES*G2, D] -> fp8, feat-major + phase-major per ragged tile:
        # out[c, f, OFFS[t]*S + k*SIZES[t] + g] = x[c*G2 + (OFFS[t]+g)*S + k, f]
        x8 = np.asarray(x).astype(NPF8).reshape(NCORES, G1, S, D)
        out = np.empty((NCORES, D, G2), NPF8)
        for t, sz in enumerate(SIZES):
            g0 = OFFS[t]
            blk = x8[:, g0:g0 + sz]                    # [C, sz, S, D]
            out[:, :, g0 * S:(g0 + sz) * S] = (
                blk.transpose(0, 3, 2, 1).reshape(NCORES, D, sz * S)
            )
        return out

    big = {
        "nn_s": shard_nn(src_neg_neg),
        "nn_d": shard_nn(dst_neg_neg),
        "neg_s": shardT(src_neg, NPBF, G1),
        "neg_d": shardT(dst_neg, NPBF, G1),
        "seed_s": shardT(src, NPF16, BL),
        "seed_d": shardT(dst, NPF16, BL),
    }
    in_maps = []
    for c in range(NCORES):
        m = dict(rep)
        for k, v in big.items():
            m[k] = v[c]
        in_maps.append(m)

    res = run_bass_kernel_spmd(
        nc, in_maps, list(range(NCORES)), trace=_trace, **trace_kwargs
    )
    out = np.concatenate([res.results[c]["out"] for c in range(NCORES)], axis=0)
    if _trace:
        return out, res
    return out


# revision 15
# speedup vs baseline: 1.2392x; 1.0829x over previous
"""GraphSAGE supervised forward on 8 Trainium2 NeuronCores.

Full inputs in, full output out. Data-parallel over the B=1024 seed nodes:
128 seeds per core; neighbor rows shard as contiguous row ranges. Tiny
weights replicated.

v6 design — quantize + transpose on host, PE group-sums, algebraic fold:
  - hop-2 neighbors (the 82MB/core f32 stream) are sent as fp8e4m3 in
    feat-major, PHASE-MAJOR-per-tile layout; hop-1 neighbors as bf16
    [128, 3200]; seeds f32. End-to-end max rel err ~2e-3 (gate 2e-2):
    the two mean-over-25 stages attenuate per-element quantization noise.
  - group-sum of 25 phases runs on the PE as accumulating identity
    matmuls: stationary [I;I] fp8 + DoubleRow packs 2 phases per
    column-slot (12 DR + 1 plain matmul per tile, all moving operands
    contiguous blocks), f32 PSUM accumulation.
  - key fold: the hop-1 mean commutes with the aggregator matmul, so
    per-column hidden states are never materialized. Only per-seed sums
    are kept: redS = DVE reduce of the GS PSUM (25 group-cols -> seed),
    negS = DVE group-sum of negT. Then per side
        m1T  = wtop^T negS + wbot^T redS      (25x mean_j h_j, transposed)
        oT   = wtop^T seedT + wbot^T m1T      (hop-1 output)
    with wbot pre-scaled by 1/25 on host. This removes the per-tile hT
    matmuls + copies that serialized the v3-v5 pipelines.
  - hop-1 + 4-layer MLP + softmax (f32) in 4 parts as seed ranges
    complete; sides interleave per tile; ragged tiles (small first/last)
    cut pipeline fill and tail latency.
"""

import sys

for _p in ("/opt/trn_rl_repo", "/root/.axon_site/_ro/trn_rl_repo"):
    if _p not in sys.path:
        sys.path.append(_p)

import numpy as np
import ml_dtypes
from contextlib import ExitStack

import concourse.bass as bass
import concourse.tile as tile
from concourse import bacc, mybir
from concourse.bass_utils import run_bass_kernel_spmd

B, S, D = 1024, 25, 128
NCORES = 8
BL = B // NCORES          # 128 seeds per core
G1 = BL * S               # 3200 hop-1 rows (= hop-2 groups) per core
G2 = G1 * S               # 80000 hop-2 rows per core

# ragged stream tiles (groups per tile, per side); sum = G1
SIZES = [100, 400, 400, 400, 400, 400, 400, 400, 200, 100]
OFFS = np.cumsum([0] + SIZES).tolist()
NTT = len(SIZES)
assert OFFS[-1] == G1 and all(sz % S == 0 for sz in SIZES)

F32 = mybir.dt.float32
F16 = mybir.dt.float16
BF16 = mybir.dt.bfloat16
F8 = mybir.dt.float8e4
AX = mybir.AxisListType
AF = mybir.ActivationFunctionType
DR = mybir.MatmulPerfMode.DoubleRow

NPF8 = ml_dtypes.float8_e4m3
NPBF = ml_dtypes.bfloat16
NPF16 = np.float16


def _build_program():
    nc = bacc.Bacc("TRN2", target_bir_lowering=False, debug=False)

    ins = {}
    for side in ("s", "d"):
        ins[f"seed_{side}"] = nc.dram_tensor(f"seed_{side}", [D, BL], F16, kind="ExternalInput")
        ins[f"neg_{side}"] = nc.dram_tensor(f"neg_{side}", [D, G1], F8, kind="ExternalInput")
        ins[f"nn_{side}"] = nc.dram_tensor(f"nn_{side}", [D, G2], F8, kind="ExternalInput")
    for name, shape, dt in (
        ("ident2", [D, 2 * D], F8),
        ("wtopH", [D, D], F16), ("wbotH", [D, D], F16),
        ("w1t", [D, D], F16), ("w1b", [D, D], F16),
        ("w2m", [D, 64], F16), ("w3m", [64, 8], F16), ("w4m", [8, 2], F16),
    ):
        ins[name] = nc.dram_tensor(name, shape, dt, kind="ExternalInput")
    out_dram = nc.dram_tensor("out", [BL, 2], F32, kind="ExternalOutput")

    with tile.TileContext(nc) as tc, ExitStack() as ctx:
        const = ctx.enter_context(tc.tile_pool(name="const", bufs=1))
        persist = ctx.enter_context(tc.tile_pool(name="persist", bufs=1))
        stream = ctx.enter_context(tc.tile_pool(name="stream", bufs=6))
        work = ctx.enter_context(tc.tile_pool(name="work", bufs=3))
        psA = ctx.enter_context(tc.tile_pool(name="psA", bufs=4, space="PSUM"))
        psM = ctx.enter_context(tc.tile_pool(name="psM", bufs=4, space="PSUM"))

        def load_const(name, shape, dt):
            t = const.tile(shape, dt, tag=name, name=name)
            nc.gpsimd.dma_start(t[:], ins[name].ap())
            return t

        # order matters: ident2 feeds the first stream tile's matmuls;
        # the big negT loads go last so they don't starve early stream tiles
        ident2 = load_const("ident2", [D, 2 * D], F8)
        negT, seedT = {}, {}
        for side in ("s", "d"):
            seedT[side] = const.tile([D, BL], F16, tag=f"seedT_{side}", name=f"seedT_{side}")
            nc.gpsimd.dma_start(seedT[side][:], ins[f"seed_{side}"].ap())
        wtopH = load_const("wtopH", [D, D], F16)
        wbotH = load_const("wbotH", [D, D], F16)
        w1t = load_const("w1t", [D, D], F16)
        w1b = load_const("w1b", [D, D], F16)
        w2m = load_const("w2m", [D, 64], F16)
        w3m = load_const("w3m", [64, 8], F16)
        w4m = load_const("w4m", [8, 2], F16)
        for side in ("s", "d"):
            negT[side] = const.tile([D, G1], F8, tag=f"negT_{side}", name=f"negT_{side}")
            nc.gpsimd.dma_start(negT[side][:], ins[f"neg_{side}"].ap())

        idv2 = ident2.rearrange("p (j m) -> p j m", j=2)  # [128, 2, 128]
        id1 = ident2[:, 0:D]                              # [128, 128]

        negS, redS = {}, {}
        for side in ("s", "d"):
            negS[side] = persist.tile([D, BL], F16, tag=f"negS_{side}", name=f"negS_{side}")
            redS[side] = persist.tile([D, BL], F16, tag=f"redS_{side}", name=f"redS_{side}")

        oT = {}
        PARTS = [(0, 52), (52, 84), (84, 116), (116, BL)]

        def hop1_m1(side, pi):
            lo, hi = PARTS[pi]
            w = hi - lo
            # m1T = wtop^T negS + wbot^T redS  (= 25x transposed mean_j h_j)
            ps_m = psM.tile([D, w], F32, tag="ps_m")
            nc.tensor.matmul(ps_m[:], wtopH[:], negS[side][:, lo:hi], start=True, stop=False)
            nc.tensor.matmul(ps_m[:], wbotH[:], redS[side][:, lo:hi], start=False, stop=True)
            m1 = work.tile([D, w], F16, tag=f"m1_{side}")
            nc.scalar.activation(m1[:], ps_m[:], AF.Copy)
            oT[side, pi, "m1"] = m1

        def hop1_o(side, pi):
            lo, hi = PARTS[pi]
            w = hi - lo
            # oT = wtop^T seedT + wbot^T m1T
            ps_o = psM.tile([D, w], F32, tag="ps_m")
            nc.tensor.matmul(ps_o[:], wtopH[:], seedT[side][:, lo:hi], start=True, stop=False)
            nc.tensor.matmul(ps_o[:], wbotH[:], oT[side, pi, "m1"][:], start=False, stop=True)
            ot = persist.tile([D, w], F16, tag=f"oT_{side}{pi}")
            nc.scalar.activation(ot[:], ps_o[:], AF.Copy)
            oT[side, pi] = ot

        mst = {}

        def mlp_l1(pi):
            lo, hi = PARTS[pi]
            w = hi - lo
            ps1 = psM.tile([D, w], F32, tag="ps_m")
            nc.tensor.matmul(ps1[:], w1t[:], oT["s", pi][:], start=True, stop=False)
            nc.tensor.matmul(ps1[:], w1b[:], oT["d", pi][:], start=False, stop=True)
            h1 = work.tile([D, w], F16, tag="h1")
            nc.scalar.activation(h1[:], ps1[:], AF.Relu)
            mst[pi, 1] = h1

        def mlp_l23(pi):
            lo, hi = PARTS[pi]
            w = hi - lo
            ps2 = psM.tile([64, w], F32, tag="ps_m")
            nc.tensor.matmul(ps2[:], w2m[:], mst[pi, 1][:])
            h2 = work.tile([64, w], F16, tag="h2")
            nc.scalar.activation(h2[:], ps2[:], AF.Relu)
            ps3 = psM.tile([8, w], F32, tag="ps_m")
            nc.tensor.matmul(ps3[:], w3m[:], h2[:])
            h3 = work.tile([8, w], F16, tag="h3")
            nc.scalar.activation(h3[:], ps3[:], AF.Relu)
            mst[pi, 3] = h3

        def mlp_sm(pi):
            lo, hi = PARTS[pi]
            w = hi - lo
            ps4 = psM.tile([w, 2], F32, tag="ps_m")
            nc.tensor.matmul(ps4[:], mst[pi, 3][:], w4m[:])
            lg = work.tile([w, 2], F32, tag="lg")
            nc.scalar.activation(lg[:], ps4[:], AF.Copy)
            nm = work.tile([w, 1], F32, tag="nm")
            nc.vector.reduce_max(nm[:], lg[:], axis=AX.X, negate=True)
            ex = work.tile([w, 2], F32, tag="ex")
            se = work.tile([w, 1], F32, tag="se")
            nc.scalar.activation(ex[:], lg[:], AF.Exp, bias=nm[:], accum_out=se[:])
            rc = work.tile([w, 1], F32, tag="rc")
            nc.vector.reciprocal(rc[:], se[:])
            o = work.tile([w, 2], F32, tag="o")
            nc.vector.tensor_scalar_mul(o[:], ex[:], rc[:])
            # SWDGE: a sync-queue store would head-of-line block stream loads
            nc.gpsimd.dma_start(out_dram.ap()[lo:hi], o[:])

        seeds_done = {"s": 0, "d": 0}
        next_part = [0]
        pieces = []

        def maybe_parts():
            # enqueue part pieces once both sides' seed sums reach a boundary;
            # pieces are drained 2 per stream tile so the in-order PE queue
            # never sits behind a long dependent chain
            while next_part[0] < len(PARTS) and min(seeds_done.values()) >= PARTS[next_part[0]][1]:
                pi = next_part[0]
                pieces.extend([
                    lambda p=pi: hop1_m1("s", p),
                    lambda p=pi: hop1_m1("d", p),
                    lambda p=pi: hop1_o("s", p),
                    lambda p=pi: hop1_o("d", p),
                    lambda p=pi: mlp_l1(p),
                    lambda p=pi: mlp_l23(p),
                    lambda p=pi: mlp_sm(p),
                ])
                next_part[0] += 1

        def drain_pieces(k):
            for _ in range(k):
                if pieces:
                    pieces.pop(0)()

        def stream_tile(side, t):
            g0, sz = OFFS[t], SIZES[t]
            xt = stream.tile([D, sz * S], F8, tag="xt", name="xt")
            nc.sync.dma_start(xt[:], ins[f"nn_{side}"].ap()[:, g0 * S:(g0 + sz) * S])
            # phase-major tile: xr[:, k, :] = phase k's sz group-columns
            xr = xt.rearrange("p (k g) -> p k g", k=S)
            ps = psA.tile([D, sz], F32, tag="ps_red")
            for i in range(S // 2):
                nc.tensor.matmul(
                    ps[:], idv2, xr[:, 2 * i:2 * i + 2, :],
                    start=(i == 0), stop=False, perf_mode=DR,
                )
            nc.tensor.matmul(ps[:], id1, xr[:, S - 1, :], start=False, stop=True)
            # per-seed sums straight from PSUM (25 group-cols per seed);
            # fp16 out: one rounding of an f32 sum, feeds the 5x-attenuated
            # mean half of hop-1
            with nc.allow_low_precision(reason="fp16 out of f32 psum sums"):
                nc.vector.reduce_sum(
                    redS[side][:, g0 // S:(g0 + sz) // S],
                    ps.rearrange("p (b s) -> p b s", s=S),
                    axis=AX.X,
                )
            seeds_done[side] = (g0 + sz) // S

        for t in range(NTT):
            for side in ("s", "d"):
                stream_tile(side, t)
                if t == 1:
                    # per-side group-sum of negT while the pipeline fills
                    with nc.allow_low_precision(reason="fp16 out of fp8 sums"):
                        nc.vector.reduce_sum(
                            negS[side][:],
                            negT[side].rearrange("p (b s) -> p b s", s=S),
                            axis=AX.X,
                        )
                maybe_parts()
                drain_pieces(2)
        while pieces:
            drain_pieces(1)

    nc.compile()
    return nc


_NC_CACHE = None


def _get_program():
    global _NC_CACHE
    if _NC_CACHE is None:
        _NC_CACHE = _build_program()
    return _NC_CACHE


def kernel(src, src_neg, src_neg_neg, dst, dst_neg, dst_neg_neg, w2, W1, W2, W3, W4,
           _trace=False, **trace_kwargs):
    nc = _get_program()

    w2 = np.asarray(w2, np.float32)
    W1 = np.asarray(W1, np.float32)
    wtop = np.ascontiguousarray(w2[:D])
    wbot = np.ascontiguousarray(w2[D:]) / np.float32(S)
    eye = np.eye(D, dtype=np.float32)
    rep = {
        "ident2": np.concatenate([eye, eye], axis=1).astype(NPF8),
        "wtopH": wtop.astype(NPF16), "wbotH": wbot.astype(NPF16),
        "w1t": np.ascontiguousarray(W1[:D]).astype(NPF16),
        "w1b": np.ascontiguousarray(W1[D:]).astype(NPF16),
        "w2m": np.asarray(W2, NPF16),
        "w3m": np.asarray(W3, NPF16),
        "w4m": np.asarray(W4, NPF16),
    }

    def shardT(x, dt, rows):
        # [NCORES*rows, D] -> transposed per core -> [NCORES, D, rows]
        return np.ascontiguousarray(
            np.asarray(x).astype(dt).reshape(NCORES, rows, D).transpose(0, 2, 1)
        )

    def shard_nn(x):
        # [NCORES*G2, D] -> fp8, feat-major + phase-major per ragged tile:
        # out[c, f, OFFS[t]*S + k*SIZES[t] + g] = x[c*G2 + (OFFS[t]+g)*S + k, f]
        x8 = np.asarray(x).astype(NPF8).reshape(NCORES, G1, S, D)
        out = np.empty((NCORES, D, G2), NPF8)
        for t, sz in enumerate(SIZES):
            g0 = OFFS[t]
            blk = x8[:, g0:g0 + sz]                    # [C, sz, S, D]
            out[:, :, g0 * S:(g0 + sz) * S] = (
                blk.transpose(0, 3, 2, 1).reshape(NCORES, D, sz * S)
            )
        return out

    big = {
        "nn_s": shard_nn(src_neg_neg),
        "nn_d": shard_nn(dst_neg_neg),
        "neg_s": shardT(src_neg, NPF8, G1),
        "neg_d": shardT(dst_neg, NPF8, G1),
        "seed_s": shardT(src, NPF16, BL),
        "seed_d": shardT(dst, NPF16, BL),
    }
    in_maps = []
    for c in range(NCORES):
        m = dict(rep)
        for k, v in big.items():
            m[k] = v[c]
        in_maps.append(m)

    res = run_bass_kernel_spmd(
        nc, in_maps, list(range(NCORES)), trace=_trace, **trace_kwargs
    )
    out = np.concatenate([res.results[c]["out"] for c in range(NCORES)], axis=0)
    if _trace:
        return out, res
    return out
